# revision 1
# baseline (speedup 1.0000x reference)
"""CLSTM-with-projection Trainium2 kernel, 8-core tensor-parallel.

Math vs the reference:
  - The char-CNN gate is an exact identity ((1-g)*x + g*x == x), so cnn_x,
    w_char, b_char never affect the output and are ignored.
  - gates(t) = x_t @ w_x + b + m(t-1) @ w_m, m = mpre @ w_proj. By
    associativity m(t-1) @ w_m = mpre(t-1) @ (w_proj @ w_m) =: mpre @ Wf.
    The recurrent state is mpre (2048), sharded 256 units/core. Every step
    each core broadcasts its feature-major state slice to the 7 peers
    (remote_dma_broadcast, sender-indexed slot via partition_id register
    offset) and accumulates all 8 slices against its Wf block (its 1024
    gate columns).
  - The output m(t) = mpre(t) @ w_proj is off the recurrent critical path:
    gathered states are stored to DRAM during the loop and projected
    afterwards, TS/8 timesteps per core.

Phases: 0) load weights, build bias tile + Wf on device. 1) xproj[t] =
x_t @ w_x + b for all t (batched, fp32r). 2) the TS-step recurrence.
3) output projection + store.

Cross-engine sync uses one monotone chain semaphore per compute engine
(pchn/achn/dvch, +1 per instruction, thresholds precomputed analytically)
plus per-slot DMA-completion semaphores (HWDGE completions can reorder
across queues, so cumulative counts are only trusted per buffer slot or
as all-done barriers).
"""

import numpy as np

import concourse.bass as bass
import concourse.mybir as mybir
from concourse import bacc, library_config

B = 128
T = 128
DIN = 512
U = 2048
PJ = 512
G = 4 * U
NC = 8
UL = U // NC  # 256
GL = G // NC  # 1024
PJL = PJ // NC  # 64
F32 = mybir.dt.float32
F32R = mybir.dt.float32r
SIGF = mybir.ActivationFunctionType.Sigmoid
TANHF = mybir.ActivationFunctionType.Tanh


def r(ap):
    return ap.bitcast(F32R)


def make_milestones(TS, TL, nb):
    M = {}
    # --- PE chain ---
    n = 0
    for c in range(16):
        n += 8
        M[f"pe_wf{c}"] = n
    for t in range(TS):
        n += nb + 8 + (32 if t >= 1 else 0)  # bias + xproj + gather
        M[f"pe_g{t}"] = n
        if t >= 1:
            n += 16  # projection of slices t-1 (own 64 output cols)
            M[f"pe_pj{t - 1}"] = n
        n += 2
        M[f"pe_tr{t}"] = n
    n += 16
    M[f"pe_pj{TS - 1}"] = n
    # --- ACT chain ---
    n = 0
    for t in range(TS):
        n += 4
        M[f"ac_sig{t}"] = n
        n += 1
        M[f"ac_tanh{t}"] = n
    # --- DVE chain ---
    n = 4  # ones memset+round, 2 c memsets
    M["dv_init"] = n
    for c in range(16):
        n += 1
        M[f"dv_wf{c}"] = n
    for t in range(TS):
        n += 3
        M[f"dv_c{t}"] = n
        n += 1
        M[f"dv_mp{t}"] = n
        n += 2
        M[f"dv_s0_{t}"] = n
        if t >= 1:
            n += 1
            M[f"dv_pj{t - 1}"] = n
    n += 1
    M[f"dv_pj{TS - 1}"] = n
    return M


def build(ts=T, knobs=(), has_bias=True):
    K1 = "nostore" in knobs  # no mpre stores / gating
    K2 = "nocomm" in knobs   # no remote broadcast / arrival waits
    K3 = "nochain" in knobs  # PE loop ignores ACT/DVE products
    NB = 2 if has_bias else 0
    TS = ts
    TL = TS // NC
    M = make_milestones(TS, TL, NB)
    nc = bacc.Bacc("TRN2", target_bir_lowering=False, debug=False, num_devices=NC)

    xT_d = nc.declare_dram_parameter("xT", [DIN, TS * B], F32R, isOutput=False)
    wx_d = nc.declare_dram_parameter("wx_s", [DIN, GL], F32R, isOutput=False)
    wm_d = nc.declare_dram_parameter("wm_s", [PJ, GL], F32R, isOutput=False)
    wpt_d = nc.declare_dram_parameter("wprojT", [PJ, U], F32R, isOutput=False)
    wpj_d = nc.declare_dram_parameter("wproj", [U, PJL], F32R, isOutput=False)
    b_d = nc.declare_dram_parameter("b_s", [1, GL], F32R, isOutput=False)
    id_d = nc.declare_dram_parameter("ident", [128, 128], F32R, isOutput=False)
    out_d = nc.declare_dram_parameter("out_m", [TS * B, PJL], F32, isOutput=True)


    from contextlib import ExitStack

    es = ExitStack()
    with es:
        sb = lambda n, sh: es.enter_context(nc.sbuf_tensor(n, sh, F32))
        sbr = lambda n, sh: es.enter_context(nc.sbuf_tensor(n, sh, F32R))
        ps = lambda n, sh: es.enter_context(nc.psum_tensor(n, sh, F32))
        sem = lambda n: es.enter_context(nc.semaphore(n))
        Wf = sbr("Wf", [128, 16 * GL])
        wx_sb = sbr("wx_sb", [128, 4 * GL])
        wm_sb = sbr("wm_sb", [128, 4 * GL])
        wpt_sb = sbr("wpt_sb", [128, 2 * 512])
        wpj_sb = sbr("wpj_sb", [128, 16 * PJL])
        bsb = sbr("bsb", [1, GL])
        ones = sb("ones", [1, 128])
        ones_r = sbr("ones_r", [1, 128])
        idt = sbr("idt", [128, 128])
        g0 = sbr("g0", [128, U])
        g1 = sbr("g1", [128, U])
        sg = sb("sg", [128, GL])
        tnh = sb("tnh", [128, UL])
        c0t = sb("c0", [128, UL])
        c1t = sb("c1", [128, UL])
        tA = sb("tA", [128, UL])
        tB = sb("tB", [128, UL])
        mp = sb("mp", [128, UL])
        xl = sbr("xl", [128, 2 * DIN])
        mo = sb("mo", [128, 2 * PJL])
        pg0 = ps("pg0", [128, GL])
        pg1 = ps("pg1", [128, GL])
        ptr = ps("ptr", [128, 256])
        ppj = ps("ppj", [128, PJL])
        rsem0 = sem("rsem0")
        rsem1 = sem("rsem1")
        lsem0 = sem("lsem0")
        lsem1 = sem("lsem1")
        psem = sem("psem")
        ldsem = sem("ldsem")
        wpa = sem("wpa")
        wpb = sem("wpb")
        xla = sem("xla")
        xlb = sem("xlb")
        os0 = sem("os0")
        os1 = sem("os1")
        pchn = sem("pchn")
        achn = sem("achn")
        dvch = sem("dvch")
        block = es.enter_context(nc.Block())

        gat = [g0, g1]
        rsems = [rsem0, rsem1]
        lsems = [lsem0, lsem1]
        pgs = [pg0, pg1]
        cts = [c0t, c1t]
        xlsems = [xla, xlb]
        wpsems = [wpa, wpb]
        osems = [os0, os1]


        LD0 = 26 * 16

        # ---------------- PE ----------------
        @block.tensor
        def _(pe):
            cnt = [0]

            def mm(*a, **kw):
                pe.matmul(*a, **kw).then_inc(pchn, 1)
                cnt[0] += 1

            def tr(*a):
                pe.transpose(*a).then_inc(pchn, 1)
                cnt[0] += 1

            pe.wait_ge(ldsem, LD0)
            pe.wait_ge(dvch, 2)  # ones ready (f32r)
            for c in range(16):
                pe.wait_ge(wpsems[c % 2], 64 * (c // 2 + 1))
                if c >= 2:
                    pe.wait_ge(dvch, M[f"dv_wf{c - 2}"])
                pg = pgs[c % 2]
                for kc in range(4):
                    for nt in range(2):
                        mm(pg[:, nt * 512 : (nt + 1) * 512],
                           r(wpt_sb[:, (c % 2) * 512 + kc * 128 : (c % 2) * 512 + (kc + 1) * 128]),
                           r(wm_sb[:, kc * GL + nt * 512 : kc * GL + (nt + 1) * 512]),
                           start=(kc == 0), stop=(kc == 3))
                assert cnt[0] == M[f"pe_wf{c}"]
            for t in range(TS):
                pe.wait_ge(xlsems[t % 2], 64 * (t // 2 + 1))
                if t < 2:
                    pe.wait_ge(dvch, M[f"dv_wf{14 + t}"])
                elif not K3:
                    pe.wait_ge(achn, M[f"ac_sig{t - 2}"])
                pg = pgs[t % 2]
                last = t == 0
                if has_bias:
                    for nt in range(2):
                        mm(pg[:, nt * 512 : (nt + 1) * 512], ones_r[:, :],
                           bsb[:, nt * 512 : (nt + 1) * 512], start=True, stop=False)
                for kc in range(4):
                    for nt in range(2):
                        mm(pg[:, nt * 512 : (nt + 1) * 512],
                           r(xl[:, (t % 2) * DIN + kc * 128 : (t % 2) * DIN + (kc + 1) * 128]),
                           r(wx_sb[:, kc * GL + nt * 512 : kc * GL + (nt + 1) * 512]),
                           start=(kc == 0 and not has_bias),
                           stop=(last and kc == 3))
                if t >= 1:
                    if not K2:
                        pe.wait_ge(rsems[(t - 1) % 2], 14 * ((t - 1) // 2 + 1))
                    if not K3:
                        pe.wait_ge(dvch, M[f"dv_s0_{t - 1}"])
                    gb = gat[(t - 1) % 2]
                    for j in range(NC):
                        for h in range(2):
                            cu = 2 * j + h
                            for nt in range(2):
                                mm(pg[:, nt * 512 : (nt + 1) * 512],
                                   r(gb[:, j * UL + h * 128 : j * UL + (h + 1) * 128]),
                                   r(Wf[:, cu * GL + nt * 512 : cu * GL + (nt + 1) * 512]),
                                   start=False, stop=(j == NC - 1 and h == 1))
                assert cnt[0] == M[f"pe_g{t}"]
                if t >= 1:
                    gb = gat[(t - 1) % 2]
                    if t >= 2:
                        pe.wait_ge(dvch, M[f"dv_pj{t - 2}"])  # ppj reuse
                    for cu in range(16):
                        mm(ppj[:, :],
                           r(gb[:, cu * 128 : (cu + 1) * 128]),
                           r(wpj_sb[:, cu * PJL : (cu + 1) * PJL]),
                           start=(cu == 0), stop=(cu == 15))
                    assert cnt[0] == M[f"pe_pj{t - 1}"]
                if not K3:
                    pe.wait_ge(dvch, M[f"dv_mp{t}"])
                    if t >= 1:
                        pe.wait_ge(dvch, M[f"dv_s0_{t - 1}"])
                for h in range(2):
                    tr(ptr[:, h * 128 : (h + 1) * 128], mp[:, h * 128 : (h + 1) * 128],
                       idt[:, :].bitcast(F32))
                assert cnt[0] == M[f"pe_tr{t}"]
            # final projection: slices TS-1
            pe.wait_ge(rsems[(TS - 1) % 2], 14 * ((TS - 1) // 2 + 1))
            pe.wait_ge(dvch, M[f"dv_s0_{TS - 1}"])
            pe.wait_ge(dvch, M[f"dv_pj{TS - 2}"])
            gb = gat[(TS - 1) % 2]
            for cu in range(16):
                mm(ppj[:, :],
                   r(gb[:, cu * 128 : (cu + 1) * 128]),
                   r(wpj_sb[:, cu * PJL : (cu + 1) * PJL]),
                   start=(cu == 0), stop=(cu == 15))
            assert cnt[0] == M[f"pe_pj{TS - 1}"]

        # ---------------- ACT ----------------
        @block.scalar
        def _(a):
            if K3:
                return
            cnt = [0]

            def act(out, in_, func, bias=0.0):
                a.wait_ge(achn, cnt[0])  # serialize same-engine
                a.activation(out, in_, func, bias=bias).then_inc(achn, 1)
                cnt[0] += 1

            for t in range(TS):
                a.wait_ge(pchn, M[f"pe_g{t}"])
                if t >= 1:
                    a.wait_ge(dvch, M[f"dv_mp{t - 1}"])
                pg = pgs[t % 2]
                act(sg[:, 0:256], pg[:, 0:256], SIGF)
                act(sg[:, 256:512], pg[:, 256:512], SIGF)
                act(sg[:, 512:768], pg[:, 512:768], SIGF, bias=1.0)
                act(sg[:, 768:1024], pg[:, 768:1024], SIGF)
                assert cnt[0] == M[f"ac_sig{t}"]
                a.wait_ge(dvch, M[f"dv_c{t}"])
                act(tnh[:, :], cts[t % 2][:, :], TANHF)
                assert cnt[0] == M[f"ac_tanh{t}"]

        # ---------------- DVE ----------------
        @block.vector
        def _(v):
            cnt = [0]

            def op(fn, *a, **kw):
                v.wait_ge(dvch, cnt[0])  # serialize same-engine
                fn(*a, **kw).then_inc(dvch, 1)
                cnt[0] += 1

            def opl(lag, fn, *a, **kw):
                # stale same-engine guard: waits an older drain point
                # (WAW vs 2 steps ago) instead of a full serialization.
                v.wait_ge(dvch, max(0, lag))
                fn(*a, **kw).then_inc(dvch, 1)
                cnt[0] += 1

            offv = v.partition_id() * UL
            op(v.memset, ones[:, :], 1.0)
            op(v.tensor_copy, ones_r[:, :], ones[:, :])
            op(v.memset, cts[0][:, :], 0.0)
            op(v.memset, cts[1][:, :], 0.0)
            assert cnt[0] == M["dv_init"]
            for c in range(16):
                v.wait_ge(pchn, M[f"pe_wf{c}"])
                op(v.tensor_copy, Wf[:, c * GL : (c + 1) * GL], pgs[c % 2][:, :])
                assert cnt[0] == M[f"dv_wf{c}"]
            for t in range(TS if not K3 else 0):
                v.wait_ge(achn, M[f"ac_sig{t}"])
                pwaw = M[f"dv_s0_{t - 1}"] if t >= 1 else 0  # prior step drained
                opl(pwaw, v.tensor_mul, tA[:, :], sg[:, 0:256], sg[:, 256:512])
                opl(pwaw, v.tensor_mul, tB[:, :], sg[:, 512:768], cts[(t + 1) % 2][:, :])
                op(v.tensor_add, cts[t % 2][:, :], tA[:, :], tB[:, :])
                assert cnt[0] == M[f"dv_c{t}"]
                v.wait_ge(achn, M[f"ac_tanh{t}"])
                if t >= 1:
                    v.wait_ge(pchn, M[f"pe_tr{t - 1}"])  # mp consumed
                opl(M[f"dv_mp{t - 1}"] if t >= 1 else 0,
                    v.tensor_mul, mp[:, :], sg[:, 768:1024], tnh[:, :])
                assert cnt[0] == M[f"dv_mp{t}"]
                v.wait_ge(pchn, M[f"pe_tr{t}"])
                if t >= 2 and not K2:
                    v.wait_ge(lsems[t % 2], 16 * (t // 2))
                gb = gat[t % 2]
                swaw = M[f"dv_s0_{t - 2}"] if t >= 2 else 0
                opl(swaw, v.tensor_copy, gb[:, bass.ds(offv, 128)], ptr[:, 0:128])
                opl(swaw, v.tensor_copy, gb[:, bass.ds(offv + 128, 128)],
                    ptr[:, 128:256])
                assert cnt[0] == M[f"dv_s0_{t}"]
                if t >= 1:
                    u = t - 1
                    v.wait_ge(pchn, M[f"pe_pj{u}"])
                    if u >= 2:
                        v.wait_ge(osems[u % 2], 16 * (u // 2))
                    opl(M[f"dv_pj{u - 2}"] if u >= 2 else 0,
                        v.tensor_copy, mo[:, (u % 2) * PJL : (u % 2 + 1) * PJL],
                        ppj[:, :])
                    assert cnt[0] == M[f"dv_pj{u}"]
            u = TS - 1
            v.wait_ge(pchn, M[f"pe_pj{u}"])
            v.wait_ge(osems[u % 2], 16 * (u // 2))
            op(v.tensor_copy, mo[:, (u % 2) * PJL : (u % 2 + 1) * PJL], ppj[:, :])
            assert cnt[0] == M[f"dv_pj{u}"]

        # ---------------- Pool: remote broadcast ----------------
        @block.gpsimd
        def _(g):
            if K2:
                return
            g.load_library(library_config.remote_dma)
            offp = g.partition_id() * UL
            rdests = [None] + [(0, j) for j in range(1, NC)]

            def descgen(t):
                gb = gat[t % 2]
                g.remote_dma_broadcast(
                    gb[:, bass.ds(offp, UL)],
                    gb[:, bass.ds(offp, UL)],
                    rsems[t % 2],
                    lsems[t % 2],
                    rdests=rdests,
                ).then_inc(psem, 1)

            PREBANK = min(6, TS)
            for t0 in range(PREBANK):
                descgen(t0)
            for t in range(TS):
                g.wait_ge(psem, t + 1)
                if not K3:
                    g.wait_ge(dvch, M[f"dv_s0_{t}"])
                g.trigger_dma(count=1)
                if t + PREBANK < TS:
                    descgen(t + PREBANK)

        # ---------------- SP ----------------
        @block.sync
        def _(s):
            for kc in range(4):
                s.dma_start(wx_sb[:, kc * GL : (kc + 1) * GL],
                            wx_d[kc * 128 : (kc + 1) * 128, :]).then_inc(ldsem, 16)
            for kc in range(4):
                s.dma_start(wm_sb[:, kc * GL : (kc + 1) * GL],
                            wm_d[kc * 128 : (kc + 1) * 128, :]).then_inc(ldsem, 16)
            for c in range(16):
                s.dma_start(wpj_sb[:, c * PJL : (c + 1) * PJL],
                            wpj_d[c * 128 : (c + 1) * 128, :]).then_inc(ldsem, 16)
            s.dma_start(bsb[:, :], b_d[:, :]).then_inc(ldsem, 16)
            s.dma_start(idt[:, :], id_d[:, :]).then_inc(ldsem, 16)
            for c in range(16):
                if c >= 2:
                    s.wait_ge(pchn, M[f"pe_wf{c - 2}"])
                for kc in range(4):
                    s.dma_start(
                        wpt_sb[:, (c % 2) * 512 + kc * 128 : (c % 2) * 512 + (kc + 1) * 128],
                        wpt_d[kc * 128 : (kc + 1) * 128, c * 128 : (c + 1) * 128],
                    ).then_inc(wpsems[c % 2], 16)
            # loop: xl loads(t) first, then out store(u = t-2)  [lag 2: the
            # pj copy of u lands during step u+1, and gating loads of t on
            # dv_pj{t-1} would cycle through PE's same-step xl dependency]
            for t in range(TS + 2):
                if t < TS:
                    if t >= 2:
                        s.wait_ge(pchn, M[f"pe_g{t - 2}"])
                    for kc in range(4):
                        s.dma_start(
                            xl[:, (t % 2) * DIN + kc * 128 : (t % 2) * DIN + (kc + 1) * 128],
                            xT_d[kc * 128 : (kc + 1) * 128, t * B : (t + 1) * B],
                        ).then_inc(xlsems[t % 2], 16)
                if t >= 2:
                    u = t - 2
                    s.wait_ge(dvch, M[f"dv_pj{u}"])
                    s.dma_start(
                        out_d[u * B : (u + 1) * B, :],
                        mo[:, (u % 2) * PJL : (u % 2 + 1) * PJL],
                    ).then_inc(osems[u % 2], 16)
            s.wait_ge(os0, 16 * (TS - TS // 2))
            s.wait_ge(os1, 16 * (TS // 2))

    nc.compile()
    return nc


# ---------------------------------------------------------------------------
# Host wrapper
# ---------------------------------------------------------------------------


# ---------------------------------------------------------------------------
# SPMD runner (inlined; modeled on concourse.bass2jax.run_bass_via_pjrt)
# ---------------------------------------------------------------------------
import time
import jax
from jax.sharding import Mesh, PartitionSpec
from jax.experimental.shard_map import shard_map
from concourse.bass2jax import (
    _bass_exec_p,
    install_neuronx_cc_hook,
    partition_id_tensor,
)


class SpmdRunner:
    def __init__(self, nc, n_cores):
        install_neuronx_cc_hook()
        self.nc = nc
        self.n_cores = n_cores
        partition_name = nc.partition_id_tensor.name if nc.partition_id_tensor else None
        in_names, out_names, out_avals, zero_outs = [], [], [], []
        for alloc in nc.m.functions[0].allocations:
            if not isinstance(alloc, mybir.MemoryLocationSet):
                continue
            name = alloc.memorylocations[0].name
            if alloc.kind == "ExternalInput":
                if name != partition_name:
                    in_names.append(name)
            elif alloc.kind == "ExternalOutput":
                out_names.append(name)
                shape = tuple(alloc.tensor_shape)
                dtype = mybir.dt.np(alloc.dtype)
                out_avals.append(jax.core.ShapedArray(shape, dtype))
                zero_outs.append(np.zeros(shape, dtype))
        self.in_names = list(in_names)
        self.out_names = out_names
        self.out_avals = out_avals
        self.zero_outs = zero_outs
        n_params = len(in_names)
        all_in_names = in_names + out_names
        if partition_name is not None:
            all_in_names.append(partition_name)

        def _body(*args):
            operands = list(args)
            if partition_name is not None:
                operands.append(partition_id_tensor())
            outs = _bass_exec_p.bind(
                *operands,
                out_avals=tuple(out_avals),
                in_names=tuple(all_in_names),
                out_names=tuple(out_names),
                lowering_input_output_aliases=(),
                sim_require_finite=True,
                sim_require_nnan=True,
                nc=nc,
            )
            return tuple(outs)

        devices = jax.devices()[:n_cores]
        self.mesh = Mesh(np.asarray(devices), ("core",))
        in_specs = (PartitionSpec("core"),) * (n_params + len(out_names))
        out_specs = (PartitionSpec("core"),) * len(out_names)
        self.fn = jax.jit(
            shard_map(
                _body,
                mesh=self.mesh,
                in_specs=in_specs,
                out_specs=out_specs,
                check_rep=False,
            ),
            keep_unused=True,
        )

    def put_inputs(self, in_maps):
        """Device-put per-core inputs (list of dicts) + zero outputs."""
        n = self.n_cores
        sh = jax.sharding.NamedSharding(self.mesh, PartitionSpec("core"))
        args = []
        for name in self.in_names:
            concat = np.concatenate([np.asarray(in_maps[c][name]) for c in range(n)], axis=0)
            args.append(jax.device_put(concat, sh))
        for z in self.zero_outs:
            concat = np.zeros((n * z.shape[0], *z.shape[1:]), z.dtype)
            args.append(jax.device_put(concat, sh))
        return args

    def run(self, args):
        outs = self.fn(*args)
        jax.block_until_ready(outs)
        return outs

    def results(self, outs):
        res = []
        for c in range(self.n_cores):
            d = {}
            for i, name in enumerate(self.out_names):
                d[name] = np.asarray(outs[i]).reshape(
                    self.n_cores, *self.out_avals[i].shape
                )[c]
            res.append(d)
        return res

    def time_it(self, args, n_warm=2, n_rep=10):
        for _ in range(n_warm):
            self.run(args)
        ts = []
        for _ in range(n_rep):
            t0 = time.perf_counter()
            self.run(args)
            ts.append(time.perf_counter() - t0)
        return min(ts), sorted(ts)[len(ts) // 2]


_CACHE = {}


def _prep_inputs(x, w, b, w_proj, ts):
    x_tm = np.ascontiguousarray(
        np.swapaxes(np.asarray(x, np.float32), 0, 1).reshape(ts * B, DIN)
    )
    xT = np.ascontiguousarray(x_tm.T)
    w = np.asarray(w, np.float32)
    b = np.asarray(b, np.float32)
    w_proj = np.ascontiguousarray(np.asarray(w_proj, np.float32))
    wpt = np.ascontiguousarray(w_proj.T)
    ident = np.eye(128, dtype=np.float32)
    in_maps = []
    for k in range(NC):
        cols = np.concatenate(
            [np.arange(gq * U + k * UL, gq * U + (k + 1) * UL) for gq in range(4)]
        )
        in_maps.append(
            {
                "xT": xT,
                "wx_s": np.ascontiguousarray(w[:DIN, cols]),
                "wm_s": np.ascontiguousarray(w[DIN:, cols]),
                "wprojT": wpt,
                "wproj": np.ascontiguousarray(w_proj[:, k * PJL : (k + 1) * PJL]),
                "b_s": np.ascontiguousarray(b[cols][None, :]),
                "ident": ident,
            }
        )
    return in_maps


def kernel(x, cnn_x, w_char, b_char, w, b, w_proj):
    ts = x.shape[1]
    hb = bool(np.any(np.asarray(b)))
    key = (ts, hb)
    if key not in _CACHE:
        _CACHE[key] = SpmdRunner(build(ts, has_bias=hb), NC)
    run = _CACHE[key]
    in_maps = _prep_inputs(x, w, b, w_proj, ts)
    args = run.put_inputs(in_maps)
    outs = run.run(args)
    res = run.results(outs)
    out = np.empty((B, ts, PJ), np.float32)
    for k in range(NC):
        om = res[k]["out_m"].reshape(ts, B, PJL)
        out[:, :, k * PJL : (k + 1) * PJL] = np.swapaxes(om, 0, 1)
    return out

